# revision 9
# baseline (speedup 1.0000x reference)
"""Trainium2 Bass kernel for a binarized-conv BasicBlock (2x BinConv3x3 + BN + residual + PReLU).

Strategy (8 NeuronCores, data-parallel over batch):
  - 64 images -> 8 per core; binarized conv weights / BN / PReLU params replicated.
  - Binarized values are exactly +/-1, so fp8 matmuls are numerically exact
    (fp32 PSUM accumulation of small integers). perf_mode=DoubleRow packs the
    full 256-channel contraction into one matmul per 3x3 tap.
  - Conv3x3 as implicit GEMM: per output tile [128 Cout x 392 cols] accumulate
    9 tap matmuls reading shifted windows of a zero-padded (30x30) binarized
    activation image.
  - Precision: stage-1 z (pre-sign) is computed fully in f32 so conv2's
    binarization never flips vs the reference; the stage-1 output is then
    rounded to bf16 (magnitude rounding cannot flip a sign). x is loaded
    both as bf16 (fast first-image path for sign) and f32 (residual).
  - BatchNorm uses full-batch statistics: per-channel (mean, E[y^2]) partials
    via bn_stats fused with PSUM evacuation, one tiny [128,4] AllReduce per
    BN (A = gamma*rsqrt(var_raw + eps/s^2) with a Newton-refined rsqrt), and
    a zero-dependency AllReduce issued at kernel start absorbs the ncfw
    first-collective wakeup plus the cross-core start skew.
  - Post-BN work is one fused scalar_tensor_tensor (z = A*y + residual) and
    Prelu(z + B) on ACT (PReLU bias absorbs the BN beta). The tail spreads
    prelu across ACT and DVE (prelu = max(z, a*z) for 0<=a<1) and streams
    each finished image to HBM immediately.
  - A dummy Sqrt as the first ACT instruction pins the single activation
    table (sqrt_and_others covers Sqrt/Sign/Prelu/Copy/Identity) so there is
    no mid-kernel ACT table swap.
"""

import numpy as np
import ml_dtypes

import concourse.bacc as bacc
import concourse.mybir as mybir
import concourse.tile as tile
from concourse.tile_rust import add_dep_helper
from concourse import bass_utils

N_CORES = 8
B_FULL, C, H, W = 64, 256, 28, 28
BL = B_FULL // N_CORES  # images per core
P = 128
NB = C // P             # channel blocks
HW = H * W              # 784
PADL = 30               # padded row length
PADQ = PADL * PADL      # 900 padded image
HALF = 14 * W           # 392 columns per psum tile (half an image)
SCALE = 0.1
BN_EPS = 1e-5

F32 = mybir.dt.float32
BF16 = mybir.dt.bfloat16
FP8 = mybir.dt.float8e4
BF16_NP = np.dtype(ml_dtypes.bfloat16)
FP8_NP = np.dtype(ml_dtypes.float8_e4m3)

_CACHE: dict = {}


def _build():
    nc = bacc.Bacc("TRN2", target_bir_lowering=False, debug=False,
                   num_devices=N_CORES)
    F = mybir.ActivationFunctionType
    Op = mybir.AluOpType
    DR = mybir.MatmulPerfMode.DoubleRow

    xh_d = nc.dram_tensor("xh", [BL, C, H, W], BF16, kind="ExternalInput")
    x_d = nc.dram_tensor("x", [BL, C, H, W], F32, kind="ExternalInput")
    # weights packed [ki, tap, i, mblk, co] with channel c = i*128 + ki
    w1_d = nc.dram_tensor("w1", [P, 9, NB, NB, P], FP8, kind="ExternalInput")
    w2_d = nc.dram_tensor("w2", [P, 9, NB, NB, P], FP8, kind="ExternalInput")
    # params packed [P, 6, NB]: order (eos1,g1,be1,eos2,g2,be2)
    # where eos = BN_EPS / (SCALE*alpha)^2 so A = gamma * rsqrt(var_raw + eos)
    par_d = nc.dram_tensor("par", [P, 6, NB], F32, kind="ExternalInput")
    # av [2]: (a1, a2) prelu alphas
    a_d = nc.dram_tensor("av", [2], F32, kind="ExternalInput")
    o_d = nc.dram_tensor("o", [BL, C, H, W], F32, kind="ExternalOutput")

    with tile.TileContext(nc) as tc:
        with (
            tc.tile_pool(name="sbuf", bufs=1) as sbuf,
            tc.tile_pool(name="psum", bufs=8, space="PSUM") as psum_pool,
            tc.tile_pool(name="dram", bufs=1, space="DRAM") as dram,
        ):
            # ---- collective warm-up: zero-dependency AllReduce issued first
            # so the ncfw first-collective wakeup burns while conv1 runs
            wz = sbuf.tile([P, 4], F32, name="wz")
            nc.vector.memset(wz[:], 0.0)
            warm_in = dram.tile([P, 4], F32, name="warm_in")
            warm_out = dram.tile([P, 4], F32, name="warm_out",
                                 addr_space="Shared")
            nc.gpsimd.dma_start(warm_in[:], wz[:])
            nc.gpsimd.collective_compute(
                "AllReduce", Op.add,
                replica_groups=[list(range(N_CORES))],
                ins=[warm_in[:]], outs=[warm_out[:]])

            # ---- image 0's second half rides the ACT queue ahead of the
            # table load (its sign runs on DVE, so ACT isn't needed yet)
            xh_src = xh_d.rearrange("b (k p) h w -> b p k (h w)", p=P)
            xh_img = [sbuf.tile([P, NB, HW], BF16, name=f"xh{b}")
                      for b in range(BL)]
            nc.sync.dma_start(xh_img[0][:, 0], xh_src[0, :, 0])
            nc.scalar.dma_start(xh_img[0][:, 1], xh_src[0, :, 1])

            # ---- pin the ACT table: first ACT instr uses Sqrt so walrus
            # loads sqrt_and_others (covers Sign/Prelu/Copy/Identity too)
            dum = sbuf.tile([P, 2], F32, name="dum")
            nc.vector.memset(dum[:], 1.0)
            nc.scalar.activation(dum[:, 1:2], dum[:, 0:1], F.Sqrt,
                                 bias=0.0, scale=1.0)

            # ---- static parameters (gpsimd queue; sync queue is for images)
            par_sb = sbuf.tile([P, 6, NB], F32)
            nc.gpsimd.dma_start(par_sb[:], par_d[:, :, :])
            eos1, g1_sb, be1_sb = (par_sb[:, i, :] for i in range(3))
            eos2, g2_sb, be2_sb = (par_sb[:, i, :] for i in range(3, 6))
            a_sb = sbuf.tile([P, 2], F32)
            nc.gpsimd.dma_start(a_sb[:], a_d[None, :].partition_broadcast(P))
            a1c, a2c = a_sb[:, 0:1], a_sb[:, 1:2]

            # ---- activation buffers ----
            # x_img: f32 x, later reused as the y2 evacuation target
            # y_img: f32 y1 evac -> z1 in place -> final f32 output
            # oh_img: bf16 stage-1 output (residual 2 + sign-2 source)
            x_img = [sbuf.tile([P, NB, HW], F32, name=f"xf{b}")
                     for b in range(BL)]
            y_img = [sbuf.tile([P, NB, HW], F32, name=f"yy{b}")
                     for b in range(BL)]
            oh_img = [sbuf.tile([P, NB, HW], BF16, name=f"oh{b}")
                      for b in range(BL)]
            xb_img = [sbuf.tile([P, NB, PADQ], FP8, name=f"xb{b}")
                      for b in range(BL)]
            xbv = [t.rearrange("p k (r c) -> p k r c", c=PADL) for t in xb_img]
            xhv = [t.rearrange("p k (r c) -> p k r c", c=W) for t in xh_img]
            ohv = [t.rearrange("p k (r c) -> p k r c", c=W) for t in oh_img]

            # zero only the pad borders; sign() fills the interior
            for b in range(BL):
                nc.vector.memset(xbv[b][:, :, 0:30:29, :], 0.0)
                nc.vector.memset(xbv[b][:, :, 1:29, 0:30:29], 0.0)

            # image 0's sign runs on DVE via (x>=0)*2-1 so the first matmul
            # is not gated on the ACT table load.
            w1_sb = sbuf.tile([P, 9, NB, NB, P], FP8)
            w2_sb = sbuf.tile([P, 9, NB, NB, P], FP8)
            sgt = sbuf.tile([P, NB, HW], FP8, name="sgt")
            nc.sync.dma_start(w1_sb[:, 0:5], w1_d[:, 0:5])
            nc.sync.dma_start(w1_sb[:, 5:9], w1_d[:, 5:9])
            for k in range(NB):
                nc.vector.tensor_scalar(sgt[:, k, :], xh_img[0][:, k, :],
                                        0.0, None, Op.is_ge)
                nc.vector.tensor_scalar(
                    xbv[0][:, k, 1:29, 1:29],
                    sgt[:, k].rearrange("p (r c) -> p r c", c=W),
                    2.0, -1.0, Op.mult, Op.add)
            last_xh_dma = None
            for b in range(1, BL):
                last_xh_dma = nc.sync.dma_start(xh_img[b][:], xh_src[b])
                for k in range(NB):
                    nc.scalar.sign(xbv[b][:, k, 1:29, 1:29], xhv[b][:, k])
            # f32 x (residual 1) and conv2 weights stream in after the bf16
            # loads so they don't steal bandwidth from the conv1 head
            x_src = x_d.rearrange("b (k p) h w -> b p k (h w)", p=P)
            for b in range(BL):
                dma = nc.gpsimd.dma_start(x_img[b][:], x_src[b])
                add_dep_helper(dma.ins, last_xh_dma.ins, sync=True,
                               reason="f32 x load after bf16 x load")
            w2dma = nc.gpsimd.dma_start(w2_sb[:], w2_d[:, :, :, :, :])
            add_dep_helper(w2dma.ins, last_xh_dma.ins, sync=True,
                           reason="w2 load after bf16 x load")

            def conv(w_sb, y_out, st6, evac_f32, dve_evac):
                """bin-conv3x3 via DoubleRow fp8 (full 256-ch contraction per
                tap); writes raw integer conv sums + per-tile stats."""
                for b in range(BL):
                    for m in range(NB):
                        for hh in range(2):
                            ps = psum_pool.tile([P, HALF], F32, name="ps",
                                                tag="ps")
                            for t in range(9):
                                dh, dw = t // 3, t % 3
                                rhs = xbv[b][:, :,
                                             hh * 14 + dh:hh * 14 + dh + 14,
                                             dw:dw + 28]
                                nc.tensor.matmul(
                                    ps[:], w_sb[:, t, :, m, :], rhs,
                                    start=(t == 0), stop=(t == 8),
                                    perf_mode=DR)
                            dst = y_out[b][:, m, hh * HALF:(hh + 1) * HALF]
                            if dve_evac(b, hh, m):
                                nc.vector.tensor_scalar(dst, ps[:], 0.0,
                                                        None, Op.add)
                            else:
                                nc.scalar.copy(dst, ps[:])
                            idx = (b * 2 + hh) * 6
                            nc.vector.bn_stats(st6[:, m, idx:idx + 6], ps[:])

            def stats_to_ab(st6, eos, g_sb, be_sb, tagn):
                """local (mean, E[y^2]) per channel -> AllReduce(add) ->
                A = gamma * rsqrt(var_raw + eps/s^2), B = beta - mean_raw*A.
                rsqrt = sqrt(1/d) refined with one Newton step."""
                st2 = sbuf.tile([P, NB, 2], F32, name=f"st2{tagn}")
                for m in range(NB):
                    nc.vector.bn_aggr(st2[:, m], st6[:, m])
                msq = sbuf.tile([P, NB], F32, name=f"msq{tagn}")
                nc.vector.tensor_tensor(msq[:], st2[:, :, 0], st2[:, :, 0],
                                        Op.mult)
                nc.vector.tensor_tensor(st2[:, :, 1], st2[:, :, 1], msq[:],
                                        Op.add)
                cc_in = dram.tile([P, 4], F32, name=f"cin{tagn}")
                cc_out = dram.tile([P, 4], F32, name=f"cout{tagn}",
                                   addr_space="Shared")
                nc.gpsimd.dma_start(cc_in[:], st2[:, :, :])
                nc.gpsimd.collective_compute(
                    "AllReduce", Op.add,
                    replica_groups=[list(range(N_CORES))],
                    ins=[cc_in[:]], outs=[cc_out[:]])
                sg = sbuf.tile([P, 4], F32, name=f"sg{tagn}")
                nc.sync.dma_start(sg[:], cc_out[:])
                mq = sbuf.tile([P, 4], F32, name=f"mq{tagn}")
                nc.vector.tensor_scalar(mq[:], sg[:], 1.0 / N_CORES, None,
                                        Op.mult)
                mg = mq[:, 0:4:2]
                e2 = mq[:, 1:4:2]
                t0 = sbuf.tile([P, NB], F32, name=f"t0{tagn}")
                d = sbuf.tile([P, NB], F32, name=f"d{tagn}")
                r = sbuf.tile([P, NB], F32, name=f"r{tagn}")
                av = sbuf.tile([P, NB], F32, name=f"av{tagn}")
                bv = sbuf.tile([P, NB], F32, name=f"bv{tagn}")
                nc.vector.tensor_tensor(t0[:], mg, mg, Op.mult)
                nc.vector.tensor_tensor(d[:], e2, t0[:], Op.subtract)
                nc.vector.tensor_tensor(d[:], d[:], eos, Op.add)
                nc.vector.reciprocal(t0[:], d[:])
                nc.scalar.activation(r[:], t0[:], F.Sqrt, bias=0.0, scale=1.0)
                # one Newton step: r <- r * (1.5 - 0.5*d*r^2)
                nc.vector.tensor_tensor(t0[:], r[:], r[:], Op.mult)
                nc.vector.tensor_tensor(t0[:], t0[:], d[:], Op.mult)
                nc.vector.tensor_scalar(t0[:], t0[:], -0.5, 1.5, Op.mult,
                                        Op.add)
                nc.vector.tensor_tensor(r[:], r[:], t0[:], Op.mult)
                nc.vector.tensor_tensor(av[:], g_sb, r[:], Op.mult)
                nc.vector.tensor_tensor(t0[:], mg, av[:], Op.mult)
                nc.vector.tensor_tensor(bv[:], be_sb, t0[:], Op.subtract)
                return av, bv

            # ================= stage 1 =================
            st6_1 = sbuf.tile([P, NB, BL * 12], F32)
            # conv1: ACT idle -> half the evacuations; last image all-ACT so
            # DVE is free for the stats tail
            conv(w1_sb, y_img, st6_1, True,
                 dve_evac=lambda b, hh, m: hh == 1 and b < BL - 1)
            a1v, b1v = stats_to_ab(st6_1, eos1, g1_sb, be1_sb, "c1")

            # post1: z1 = A*y1 + x in f32 (DVE, in place), out1 = Prelu(z+B)
            # -> bf16 (ACT), sign -> fp8 (ACT). Image 0's m1 path runs on
            # DVE (prelu = max(a*z, z); sign via (z>=0)*2-1) so conv2's
            # first matmul unblocks ~2us earlier.
            for b in range(BL):
                for m in range(NB):
                    nc.vector.scalar_tensor_tensor(
                        y_img[b][:, m, :], y_img[b][:, m, :], a1v[:, m:m + 1],
                        x_img[b][:, m, :], Op.mult, Op.add)
                    zb = y_img[b][:, m, :]
                    if b == 0 and m == 1:
                        nc.vector.tensor_scalar(zb, zb, b1v[:, m:m + 1],
                                                None, Op.add)
                        nc.vector.scalar_tensor_tensor(
                            oh_img[b][:, m, :], zb, a1c, zb, Op.mult, Op.max)
                        nc.vector.tensor_scalar(sgt[:, m, :],
                                                oh_img[b][:, m, :],
                                                0.0, None, Op.is_ge)
                        nc.vector.tensor_scalar(
                            xbv[b][:, m, 1:29, 1:29],
                            sgt[:, m].rearrange("p (r c) -> p r c", c=W),
                            2.0, -1.0, Op.mult, Op.add)
                    else:
                        nc.scalar.activation(
                            oh_img[b][:, m, :], zb, F.Prelu,
                            bias=b1v[:, m:m + 1], scale=1.0, alpha=a1c)
                        nc.scalar.sign(xbv[b][:, m, 1:29, 1:29], ohv[b][:, m])

            # ================= stage 2 =================
            st6_2 = sbuf.tile([P, NB, BL * 12], F32)
            # ACT is busy with post1 prelus/signs early on -> DVE takes the
            # early evacuations, ACT the late ones
            conv(w2_sb, x_img, st6_2, True,
                 dve_evac=lambda b, hh, m: b < 5)
            a2v, b2v = stats_to_ab(st6_2, eos2, g2_sb, be2_sb, "c2")

            o_dst = o_d.rearrange("b (k p) h w -> b p k (h w)", p=P)
            # post2: z2 = A*y2 + out1 (DVE; GpSimd adds for 4,5), out =
            # Prelu(z + B): ACT for images 0-5, DVE max-trick for 6,7.
            # Stream each image out as soon as it is done.
            for b in range(BL):
                for m in range(NB):
                    zb = x_img[b][:, m, :]
                    if b in (4, 5):
                        nc.vector.tensor_scalar(zb, zb, a2v[:, m:m + 1],
                                                b2v[:, m:m + 1],
                                                Op.mult, Op.add)
                        nc.gpsimd.tensor_tensor(zb, zb, oh_img[b][:, m, :],
                                                Op.add)
                        nc.scalar.activation(
                            y_img[b][:, m, :], zb, F.Prelu,
                            bias=0.0, scale=1.0, alpha=a2c)
                    elif b < 4:
                        nc.vector.scalar_tensor_tensor(
                            zb, zb, a2v[:, m:m + 1], oh_img[b][:, m, :],
                            Op.mult, Op.add)
                        nc.scalar.activation(
                            y_img[b][:, m, :], zb, F.Prelu,
                            bias=b2v[:, m:m + 1], scale=1.0, alpha=a2c)
                    else:
                        nc.vector.scalar_tensor_tensor(
                            zb, zb, a2v[:, m:m + 1], oh_img[b][:, m, :],
                            Op.mult, Op.add)
                        nc.vector.tensor_scalar(zb, zb, b2v[:, m:m + 1],
                                                None, Op.add)
                        nc.vector.scalar_tensor_tensor(
                            y_img[b][:, m, :], zb, a2c, zb, Op.mult, Op.max)
                nc.sync.dma_start(o_dst[b], y_img[b][:])

    nc.compile()
    return nc


def _get_nc():
    if "nc" not in _CACHE:
        _CACHE["nc"] = _build()
    return _CACHE["nc"]


def _pack_w(w):
    wb = np.sign(np.asarray(w, np.float32))
    # [co, ci, kh, kw] -> [ki, tap, i, co_blk, co] with ci = i*128 + ki
    t = wb.reshape(NB, P, NB, P, 3, 3)
    t = np.transpose(t, (3, 4, 5, 2, 0, 1)).reshape(P, 9, NB, NB, P)
    return np.ascontiguousarray(t).astype(FP8_NP)


def _pack_par(s1, g1, be1, s2, g2, be2):
    eos1 = BN_EPS / (s1 * s1)
    eos2 = BN_EPS / (s2 * s2)
    par = np.stack([np.asarray(v, np.float32).reshape(NB, P)
                    for v in (eos1, g1, be1, eos2, g2, be2)])  # [6, NB, P]
    return np.ascontiguousarray(par.transpose(2, 0, 1).astype(np.float32))


def _make_in_maps(x, conv1_w, conv2_w, bn1_gamma, bn1_beta, bn2_gamma,
                  bn2_beta, prelu1_a, prelu2_a):
    x = np.ascontiguousarray(np.asarray(x, np.float32))
    s1 = SCALE * np.mean(np.abs(np.asarray(conv1_w, np.float32)),
                         axis=(1, 2, 3), dtype=np.float32)
    s2 = SCALE * np.mean(np.abs(np.asarray(conv2_w, np.float32)),
                         axis=(1, 2, 3), dtype=np.float32)
    a1 = float(np.asarray(prelu1_a, np.float32).reshape(()))
    a2 = float(np.asarray(prelu2_a, np.float32).reshape(()))
    shared = {
        "w1": _pack_w(conv1_w), "w2": _pack_w(conv2_w),
        "par": _pack_par(s1, bn1_gamma, bn1_beta, s2, bn2_gamma, bn2_beta),
        "av": np.array([a1, a2], np.float32),
    }
    xh = x.astype(BF16_NP)
    return [dict(shared, x=x[c * BL:(c + 1) * BL],
                 xh=xh[c * BL:(c + 1) * BL]) for c in range(N_CORES)]


def kernel(x, conv1_w, conv2_w, bn1_gamma, bn1_beta, bn2_gamma, bn2_beta,
           prelu1_a, prelu2_a):
    nc = _get_nc()
    in_maps = _make_in_maps(x, conv1_w, conv2_w, bn1_gamma, bn1_beta,
                            bn2_gamma, bn2_beta, prelu1_a, prelu2_a)
    res = bass_utils.run_bass_kernel_spmd(nc, in_maps,
                                          core_ids=list(range(N_CORES)))
    out = np.concatenate([res.results[c]["o"] for c in range(N_CORES)], axis=0)
    return out
